# revision 1
# baseline (speedup 1.0000x reference)
"""Distributed exact-KNN (L1 distance, k=16) on 8 Trainium2 NeuronCores.

Strategy (classic distributed kNN reduce):
  - Shard the 50000 train rows across 8 cores (6272 rows/core, padded).
  - Per core, a Bass/Tile kernel computes all 128 x 6272 L1 distances:
      * train shard pre-transposed on host to [128, 3136] bf16
        (partition p = 64*h + d holds dim d of row-half h),
      * DVE tensor_scalar / ACT activation compute |T - x_b| per test point
        (per-partition fp32 scalar/bias = -x_b, duplicated per half),
      * PE reduces the 64 dims per half with a stationary [128, 2] block
        matrix of -1s -> PSUM holds -dist for 64 test points per round,
      * DVE max8/max_index/match_replace extract the top-32 (smallest
        distance) candidates per (test point, shard half).
  - Host gathers 8 cores x 2 halves x 32 candidates per test point,
    re-computes their exact distances in float64, takes the global top-16
    (ties by lowest train index, matching jax.lax.top_k), sums the
    train_target rows and argmaxes -> predicted class.

The bf16 scan only needs to rank candidates to within its error (~0.1 abs
on distances ~72 with a 16-vs-17 boundary gap ~0.1 per order statistic);
the 32-deep candidate margin (16 extra order statistics ~ 1.8) makes a
miss of a true top-16 neighbor effectively impossible, and the host
refinement restores exact arithmetic for the final answer.
"""

import numpy as np

import ml_dtypes

import concourse.bass as bass
import concourse.tile as tile
from concourse import bacc, mybir
from concourse.bass_utils import run_bass_kernel_spmd

# Problem constants (hardcoded per harness contract).
N_TRAIN, D, B, N_CLASSES, K = 50000, 64, 128, 10, 16
N_CORES = 8
NSH = 6272          # train rows per core (8 * 6272 = 50176 >= 50000, padded)
HALF = NSH // 2     # 3136 columns per half
CH = 448            # PSUM chunk width (<= 512 fp32 / bank)
NCHUNK = HALF // CH  # 7
KC = 32             # candidates kept per (test point, shard half)
PAD_VAL = 1.0e4     # pad train rows -> distance ~64e4, never a candidate

_CACHE = {}


def _build_program():
    """Build the SPMD Bass program (identical on all cores)."""
    nc = bacc.Bacc(
        "TRN2",
        target_bir_lowering=False,
        debug=False,
        enable_asserts=False,
        num_devices=N_CORES,
    )
    f32 = mybir.dt.float32
    bf16 = mybir.dt.bfloat16
    u32 = mybir.dt.uint32

    a_dram = nc.dram_tensor("a", [128, HALF], bf16, kind="ExternalInput")
    xn_dram = nc.dram_tensor("xn", [128, 128], f32, kind="ExternalInput")
    w_dram = nc.dram_tensor("w", [128, 64 * 128], bf16, kind="ExternalInput")
    vals_dram = nc.dram_tensor("vals", [256, KC], f32, kind="ExternalOutput")
    idxs_dram = nc.dram_tensor("idxs", [256, KC], u32, kind="ExternalOutput")

    with tile.TileContext(nc) as tc:
        with (
            tc.tile_pool(name="const", bufs=1) as const,
            tc.tile_pool(name="v", bufs=3) as vpool,
            tc.tile_pool(name="dist", bufs=2) as dpool,
            tc.tile_pool(name="outs", bufs=2) as opool,
            tc.tile_pool(name="psum", bufs=1, space="PSUM") as ppool,
        ):
            a_sb = const.tile([128, HALF], bf16)
            nc.sync.dma_start(out=a_sb, in_=a_dram.ap())
            xn_sb = const.tile([128, 128], f32)
            nc.sync.dma_start(out=xn_sb, in_=xn_dram.ap())
            w_sb = const.tile([128, 64 * 128], bf16)
            nc.sync.dma_start(out=w_sb, in_=w_dram.ap())

            for r in range(2):  # two rounds of 64 test points
                ptiles = [
                    ppool.tile([128, CH], f32, tag=f"ps{c}", name=f"ps{c}_{r}")
                    for c in range(NCHUNK)
                ]
                for bp in range(64):
                    b = 64 * r + bp
                    v = vpool.tile([128, HALF], bf16)
                    # |a - x_b| on ACT (abs_max is not ISA-valid on DVE
                    # tensor_scalar; ACT Abs with per-partition bias is).
                    nc.scalar.activation(
                        out=v,
                        in_=a_sb,
                        func=mybir.ActivationFunctionType.Abs,
                        bias=xn_sb[:, b : b + 1],
                        scale=1.0,
                    )
                    # -dist for both halves of test point b lands in psum
                    # rows (2*bp, 2*bp+1); 64 MMs accumulate per chunk tile
                    # (weight block bp is zero outside its two columns).
                    for c in range(NCHUNK):
                        nc.tensor.matmul(
                            out=ptiles[c],
                            lhsT=w_sb[:, 128 * bp : 128 * bp + 128],
                            rhs=v[:, c * CH : (c + 1) * CH],
                            start=(bp == 0),
                            stop=(bp == 63),
                        )
                dist = dpool.tile([128, HALF], f32)
                for c in range(NCHUNK):
                    nc.vector.tensor_copy(out=dist[:, c * CH : (c + 1) * CH], in_=ptiles[c])
                # Top-KC per row (row = one (test point, half) pair).
                vals_t = opool.tile([128, KC], f32, tag="vals")
                idxs_t = opool.tile([128, KC], u32, tag="idxs")
                for t in range(KC // 8):
                    nc.vector.max(out=vals_t[:, 8 * t : 8 * t + 8], in_=dist)
                    nc.vector.max_index(
                        out=idxs_t[:, 8 * t : 8 * t + 8],
                        in_max=vals_t[:, 8 * t : 8 * t + 8],
                        in_values=dist,
                    )
                    if t < KC // 8 - 1:
                        nc.vector.match_replace(
                            out=dist,
                            in_to_replace=vals_t[:, 8 * t : 8 * t + 8],
                            in_values=dist,
                            imm_value=-3.0e38,
                        )
                nc.sync.dma_start(out=vals_dram.ap()[128 * r : 128 * (r + 1), :], in_=vals_t)
                nc.sync.dma_start(out=idxs_dram.ap()[128 * r : 128 * (r + 1), :], in_=idxs_t)
    nc.compile()
    return nc


def _prep_inputs(train_data, x_test):
    """Host-side shard prep: pad, transpose, stack halves, cast bf16."""
    padded = np.full((N_CORES * NSH, D), PAD_VAL, dtype=np.float32)
    padded[:N_TRAIN] = train_data
    xn = -x_test.T.astype(np.float32)          # [64, 128]
    xn_full = np.concatenate([xn, xn], axis=0)  # [128, 128]
    # 64 stationary weight blocks: block bp is [128, 128] with -1 in rows of
    # half h at output column 2*bp + h (accumulating matmuls place test point
    # bp's two half-distances in psum rows 2*bp, 2*bp+1).
    w = np.zeros((128, 64, 128), dtype=np.float32)
    for bp in range(64):
        w[:64, bp, 2 * bp] = -1.0
        w[64:, bp, 2 * bp + 1] = -1.0
    w_bf = w.reshape(128, 64 * 128).astype(ml_dtypes.bfloat16)
    in_maps = []
    for c in range(N_CORES):
        shard = padded[c * NSH : (c + 1) * NSH]          # [6272, 64]
        halves = shard.reshape(2, HALF, D)               # [2, 3136, 64]
        a = np.concatenate([halves[0].T, halves[1].T])   # [128, 3136]
        in_maps.append(
            {
                "a": np.ascontiguousarray(a).astype(ml_dtypes.bfloat16),
                "xn": np.ascontiguousarray(xn_full),
                "w": w_bf,
            }
        )
    return in_maps


def _run_device(train_data, x_test, trace=False):
    if "nc" not in _CACHE:
        _CACHE["nc"] = _build_program()
    nc = _CACHE["nc"]
    in_maps = _prep_inputs(train_data, x_test)
    res = run_bass_kernel_spmd(
        nc, in_maps, core_ids=list(range(N_CORES)), trace=trace
    )
    return res


def kernel(train_data, train_target, x_test, k, _trace=False, _ret_raw=False):
    train_data = np.asarray(train_data, dtype=np.float32)
    train_target = np.asarray(train_target, dtype=np.float32)
    x_test = np.asarray(x_test, dtype=np.float32)
    k = int(k)

    res = _run_device(train_data, x_test, trace=_trace)

    # Gather candidates from all cores and decode indices.
    cand_n = [[] for _ in range(B)]
    for c in range(N_CORES):
        out = res.results[c]
        idxs = out["idxs"].astype(np.int64)  # [256, KC]
        for row in range(256):
            r, p = divmod(row, 128)
            bi, h = divmod(p, 2)
            b = 64 * r + bi
            n_glob = c * NSH + h * HALF + idxs[row]
            cand_n[b].append(n_glob)

    # Exact refinement in float64 + vote.
    td = train_data.astype(np.float64)
    xt = x_test.astype(np.float64)
    preds = np.empty(B, dtype=np.int32)
    for b in range(B):
        n = np.unique(np.concatenate(cand_n[b]))
        n = n[n < N_TRAIN]
        d = np.abs(td[n] - xt[b]).sum(axis=1)
        order = np.lexsort((n, d))[:k]
        votes = train_target[n[order]].sum(axis=0)
        preds[b] = int(np.argmax(votes))

    if _ret_raw:
        return preds, res
    return preds



# revision 2
# speedup vs baseline: 7.9536x; 7.9536x over previous
"""Distributed exact-KNN (L1 distance, k=16) on 8 Trainium2 NeuronCores.

Strategy (quantized-score screening + exact host refinement):
  - Shard the 50000 train rows across 8 cores (6272 rows/core, padded).
  - Screening score: quantize each train value to a 17-level grid
    t_0..t_16 (round-to-nearest via 16 midpoint thresholds s_j).  Then
        |q(a) - x| = |t_0 - x| - sum_j 1[a > s_j] * (|t_{j-1}-x| - |t_j-x|)
    so, dropping per-test-point constants, the ranking score
        R[b, n] = sum_{d, j} Phi[(d,j), n] * M[(d,j), b],
        Phi = 1[a_nd > s_j]  (device-computed, bf16 0/1),
        M   = |t_{j-1} - x_bd| - |t_j - x_bd|  (host-computed lhsT, bf16)
    is a single dense 1024-contraction matmul; PSUM holds R for all 128
    test points (partitions) x train columns.  Maximizing R == minimizing
    the quantized L1 distance.
  - Per core: 8 contraction slices of 128 (= 64 dims x 2 features),
    encoded from a duplicated train tile a2[64r+d, n] = a[n, d] by
    one tensor_scalar(is_gt) / activation(Sign) pass per slice.
    Sign slices use M/2 since sign = 2*Phi - 1 (constants drop out).
  - Matmul in 2 waves x 7 PSUM chunks of 448 fp32; DVE max8/max_index
    extracts top-8 (value+index) per (test point, 448-col chunk) directly
    from PSUM -> 8 cores x 14 chunks x 8 = 896 candidates per test point.
  - Host: exact fp64 distances for candidates, global top-k with
    tie-break by lowest index (matches jax.lax.top_k), vote, argmax.
  Numpy-validated on the real data: every true top-16 neighbor ranks
  <= 2 within its 448-chunk (we keep 8) -> exactness margin is large.
"""

import numpy as np

import ml_dtypes

import concourse.bass as bass
import concourse.tile as tile
from concourse import bacc, mybir
from concourse.bass_utils import run_bass_kernel_spmd

# Problem constants (hardcoded per harness contract).
N_TRAIN, D, B, N_CLASSES = 50000, 64, 128, 10
N_CORES = 8
NSH = 6272           # train rows per core (8 * 6272 = 50176 >= 50000, padded)
NW = 2               # column waves
WCOLS = NSH // NW    # 3136
CH = 448             # PSUM chunk width (1792 B < one 2 KiB bank)
NCHW = WCOLS // CH   # 7 chunks per wave
NCHUNK = NW * NCHW   # 14
NLEV = 17            # quantization levels t_0..t_16
NFEAT = NLEV - 1     # 16 threshold features per dim
NSLICE = NFEAT // 2  # 8 matmul contraction slices (64 dims x 2 features)
LO, HI = -2.6, 2.6
PAD_VAL = 1.0e4      # pad train rows quantize to t_16, score ~94 below real
ACT_SLICES = (4, 5, 6, 7)  # encoded on ScalarE (Sign); rest DVE (is_gt)
CENTER = 94.0        # score centering (numerics only; scores are fp32)

_CACHE = {}


def _build_program():
    """Build the SPMD Bass program (identical on all cores)."""
    nc = bacc.Bacc(
        "TRN2",
        target_bir_lowering=False,
        debug=False,
        enable_asserts=False,
        num_devices=N_CORES,
    )
    f32 = mybir.dt.float32
    bf16 = mybir.dt.bfloat16
    u16 = mybir.dt.uint16

    a2_dram = nc.dram_tensor("a2", [128, NSH], bf16, kind="ExternalInput")
    w_dram = nc.dram_tensor("w", [128, NSLICE * 128], bf16, kind="ExternalInput")
    sv_dram = nc.dram_tensor("sv", [128, NSLICE], f32, kind="ExternalInput")
    nsv_dram = nc.dram_tensor("nsv", [128, NSLICE], f32, kind="ExternalInput")
    vals_dram = nc.dram_tensor("vals", [128, NCHUNK * 8], f32, kind="ExternalOutput")
    idxs_dram = nc.dram_tensor("idxs", [128, NCHUNK * 8], u16, kind="ExternalOutput")

    with tile.TileContext(nc) as tc:
        with (
            tc.tile_pool(name="const", bufs=1) as const,
            tc.tile_pool(name="phi", bufs=1) as phipool,
            tc.tile_pool(name="outs", bufs=1) as opool,
            tc.tile_pool(name="psum", bufs=1, space="PSUM") as ppool,
        ):
            aw = []
            for w in range(NW):
                t = const.tile([128, WCOLS], bf16, tag=f"a{w}")
                nc.sync.dma_start(out=t, in_=a2_dram.ap()[:, w * WCOLS : (w + 1) * WCOLS])
                aw.append(t)
            w_sb = const.tile([128, NSLICE * 128], bf16, tag="w")
            nc.sync.dma_start(out=w_sb, in_=w_dram.ap())
            sv_sb = const.tile([128, NSLICE], f32, tag="sv")
            nc.sync.dma_start(out=sv_sb, in_=sv_dram.ap())
            nsv_sb = const.tile([128, NSLICE], f32, tag="nsv")
            nc.sync.dma_start(out=nsv_sb, in_=nsv_dram.ap())

            # Threshold encode: phi[(s, w)] = 1[a > s_j] (or sign thereof).
            phi = {}
            for w in range(NW):
                for s in range(NSLICE):
                    t = phipool.tile([128, WCOLS], bf16, tag=f"phi{s}_{w}")
                    if s in ACT_SLICES:
                        nc.scalar.activation(
                            out=t,
                            in_=aw[w],
                            func=mybir.ActivationFunctionType.Sign,
                            bias=nsv_sb[:, s : s + 1],
                            scale=1.0,
                        )
                    else:
                        nc.vector.tensor_scalar(
                            out=t,
                            in0=aw[w],
                            scalar1=sv_sb[:, s : s + 1],
                            scalar2=None,
                            op0=mybir.AluOpType.is_gt,
                        )
                    phi[(s, w)] = t

            vals_sb = opool.tile([128, NCHUNK * 8], f32, tag="vals")
            idxs_sb = opool.tile([128, NCHUNK * 8], u16, tag="idxs")
            for w in range(NW):
                ptiles = [
                    ppool.tile([128, CH], f32, tag=f"ps{c}", name=f"ps{c}_{w}")
                    for c in range(NCHW)
                ]
                for s in range(NSLICE):
                    for c in range(NCHW):
                        nc.tensor.matmul(
                            out=ptiles[c],
                            lhsT=w_sb[:, 128 * s : 128 * (s + 1)],
                            rhs=phi[(s, w)][:, CH * c : CH * (c + 1)],
                            start=(s == 0),
                            stop=(s == NSLICE - 1),
                        )
                for c in range(NCHW):
                    g = w * NCHW + c
                    nc.vector.max(out=vals_sb[:, 8 * g : 8 * g + 8], in_=ptiles[c])
                    nc.vector.max_index(
                        out=idxs_sb[:, 8 * g : 8 * g + 8],
                        in_max=vals_sb[:, 8 * g : 8 * g + 8],
                        in_values=ptiles[c],
                    )
            nc.sync.dma_start(out=vals_dram.ap(), in_=vals_sb)
            nc.sync.dma_start(out=idxs_dram.ap(), in_=idxs_sb)
    nc.compile()
    return nc


def _prep_inputs(train_data, x_test):
    """Host-side prep: quantization grid, per-core duplicated train tiles,
    per-test-point delta tables (lhsT), threshold vectors."""
    levels = np.linspace(LO, HI, NLEV).astype(np.float32)       # t_0..t_16
    thr = ((levels[:-1] + levels[1:]) / 2).astype(np.float32)   # s_1..s_16

    # lhsT: w[64r+d, 128s+b] = M[d, f](b), f = 2s+r
    #   M[d, f](b) = |t_f - x_bd| - |t_{f+1} - x_bd|
    Mtab = np.abs(levels[:-1][None, :, None] - x_test.T[:, None, :]) - np.abs(
        levels[1:][None, :, None] - x_test.T[:, None, :]
    )  # [D, NFEAT, B]
    w = np.empty((128, NSLICE, B), dtype=np.float32)
    for s in range(NSLICE):
        scale = 0.5 if s in ACT_SLICES else 1.0
        w[:64, s, :] = Mtab[:, 2 * s, :] * scale
        w[64:, s, :] = Mtab[:, 2 * s + 1, :] * scale
    w_bf = np.ascontiguousarray(w.reshape(128, NSLICE * B)).astype(
        ml_dtypes.bfloat16
    )

    sv = np.empty((128, NSLICE), dtype=np.float32)
    for s in range(NSLICE):
        sv[:64, s] = thr[2 * s]
        sv[64:, s] = thr[2 * s + 1]
    nsv = np.ascontiguousarray(-sv)

    padded = np.full((N_CORES * NSH, D), PAD_VAL, dtype=np.float32)
    padded[:N_TRAIN] = train_data
    in_maps = []
    for c in range(N_CORES):
        shard_t = padded[c * NSH : (c + 1) * NSH].T  # [64, 6272]
        a2 = np.concatenate([shard_t, shard_t], axis=0)  # [128, 6272]
        in_maps.append(
            {
                "a2": np.ascontiguousarray(a2).astype(ml_dtypes.bfloat16),
                "w": w_bf,
                "sv": sv,
                "nsv": nsv,
            }
        )
    return in_maps


def _run_device(train_data, x_test, trace=False):
    if "nc" not in _CACHE:
        _CACHE["nc"] = _build_program()
    nc = _CACHE["nc"]
    in_maps = _prep_inputs(train_data, x_test)
    res = run_bass_kernel_spmd(
        nc, in_maps, core_ids=list(range(N_CORES)), trace=trace
    )
    return res


def kernel(train_data, train_target, x_test, k, _trace=False, _ret_raw=False):
    train_data = np.asarray(train_data, dtype=np.float32)
    train_target = np.asarray(train_target, dtype=np.float32)
    x_test = np.asarray(x_test, dtype=np.float32)
    k = int(k)

    res = _run_device(train_data, x_test, trace=_trace)

    # Candidate decode: chunk g covers shard cols [448g, 448g+448).
    base = (np.arange(NCHUNK) * CH).repeat(8)[None, :]  # [1, 112]
    cand = np.empty((B, N_CORES * NCHUNK * 8), dtype=np.int64)
    for c in range(N_CORES):
        idxs = res.results[c]["idxs"].astype(np.int64)  # [128, 112]
        cand[:, c * NCHUNK * 8 : (c + 1) * NCHUNK * 8] = c * NSH + base + idxs

    # Exact refinement in float64 + vote (tie-break by lowest index).
    td = train_data.astype(np.float64)
    xt = x_test.astype(np.float64)
    preds = np.empty(B, dtype=np.int32)
    for b in range(B):
        n = np.unique(cand[b])
        n = n[n < N_TRAIN]
        d = np.abs(td[n] - xt[b]).sum(axis=1)
        order = np.lexsort((n, d))[:k]
        votes = train_target[n[order]].sum(axis=0)
        preds[b] = int(np.argmax(votes))

    if _ret_raw:
        return preds, res
    return preds
